# revision 10
# baseline (speedup 1.0000x reference)
"""DETM nelbo kernel for 8 Trainium2 NeuronCores (v2, all-bf16, SBUF-resident).

Sharding: vocabulary V=30000 split 8 ways (3750/core).
 - theta-MLP first layer contracts over V -> per-core partial + AllReduce.
 - beta path: logit[t,k,v] = alphas.rho per V-slice; exp(logit) kept RESIDENT
   in SBUF as bf16 (no DRAM round trip), processed in 4 splits of 6x128 TK
   rows with ping-pong buffers. Z partials AllReduced per split (overlapped
   with the next split's einsum).
 - G[r, b] = theta[b, k(r)] * (times[b]==t(r)) / Z[r] built on device per
   split (Psel matmul + two DVE muls); mix[b,v] accumulated via PE matmuls
   into an SBUF f32 accumulator; nll = -sum bows*ln(mix+1e-6) per V-slice.
Small sequential chains (alpha reparam + KLs, LSTM, eta chain) replicated on
the host in fp32 numpy.
"""
import sys

if "/opt/trn_rl_repo" not in sys.path:
    sys.path.insert(0, "/opt/trn_rl_repo")

import numpy as np
import ml_dtypes

import concourse.bass as bass
import concourse.mybir as mybir
import concourse.tile as tile
from concourse import bacc, bass_utils
from concourse.masks import make_identity

F32 = mybir.dt.float32
BF16 = mybir.dt.bfloat16
AF = mybir.ActivationFunctionType
OP = mybir.AluOpType
BFNP = ml_dtypes.bfloat16

V, K, E, T, B = 30000, 50, 300, 60, 128
TH, H, L = 800, 200, 3
NCORES = 8
VS = V // NCORES          # 3750
TK = T * K                # 3000
TKP = 3072                # padded to 24 chunks of 128
MCH = 24                  # TK chunks of 128 rows
NSPL = 4                  # splits
MS = MCH // NSPL          # 6 chunks per split
NW4 = [1024, 1024, 1024, 678]   # V-chunking of VS=3750
DELTA = 0.005

EK = [128, 128, 44]       # E=300 partition split
KJ = [128, 128, 128, 128, 128, 128, 32]   # TH=800 split

_CACHE = {}


def _build_program():
    nc = bacc.Bacc("TRN2", target_bir_lowering=False, debug=False,
                   num_devices=NCORES)

    def din(name, shape, dt=F32):
        return nc.dram_tensor(name, shape, dt, kind="ExternalInput").ap()

    nbT = din("nbT", [VS, B], BF16)
    w1vT = din("w1vT", [VS, TH], BF16)
    rhoT = din("rhoT", [E, VS], BF16)
    alphasT = din("alphasT", [E, TKP], BF16)
    bowsS = din("bowsS", [B, VS])
    maskB = din("maskB", [128, MCH * B], BF16)
    pselB = din("pselB", [K, MCH * 128], BF16)
    etaC = din("etaC", [B, TH])
    etaTD = din("etaTD", [B, K])
    epsTH = din("epsTH", [B, K])
    w2T = din("w2T", [TH, TH], BF16)
    wmulsT = din("wmulsT", [TH, 2 * K], BF16)
    b2R = din("b2R", [128, 7])
    bmulsB = din("bmulsB", [B, 2 * K])

    nllOut = nc.dram_tensor("nllOut", [B, 1], F32, kind="ExternalOutput").ap()
    klthOut = nc.dram_tensor("klthOut", [B, 1], F32, kind="ExternalOutput").ap()
    canOut = nc.dram_tensor("canOut", [128, 64], F32, kind="ExternalOutput").ap()

    RKL = np.float32(0.5 / (1.0 + 1e-6))

    with tile.TileContext(nc) as tc:
        with tc.tile_pool(name="outer", bufs=1) as outer, \
             tc.tile_pool(name="sp", bufs=2) as sp, \
             tc.tile_pool(name="wrk", bufs=2) as wrk, \
             tc.tile_pool(name="dramp", bufs=1, space="DRAM") as dram, \
             tc.tile_pool(name="peps", bufs=2, space="PSUM") as peps, \
             tc.tile_pool(name="pmps", bufs=2, space="PSUM") as pmps:

            ident = outer.tile([128, 128], F32)
            make_identity(nc, ident[:])
            mixacc = outer.tile([128, VS], F32)
            nc.vector.memset(mixacc[:], 0.0)
            eps6 = outer.tile([B, 1], F32)
            nc.vector.memset(eps6[:], 1e-6)

            # persistent input DMAs (needed early)
            rho_sb = outer.tile([128, 3, VS], BF16)
            for kc in range(3):
                nc.sync.dma_start(rho_sb[:EK[kc], kc, :],
                                  rhoT[kc * 128:kc * 128 + EK[kc], :])
            masks_sb = outer.tile([128, MCH, B], BF16)
            nc.sync.dma_start(masks_sb[:],
                              maskB[:].rearrange("p (m b) -> p m b", b=B))
            psel_sb = outer.tile([K, MCH, 128], BF16)
            nc.sync.dma_start(psel_sb[:],
                              pselB[:].rearrange("p (m b) -> p m b", b=128))

            ar1_in = dram.tile([B, TH], F32)
            ar1_out = dram.tile([B, TH], F32, addr_space="Shared")
            z_in = [dram.tile([MS * 128], F32, name=f"zin{s}")
                    for s in range(NSPL)]
            z_out = [dram.tile([MS * 128], F32, addr_space="Shared",
                               name=f"zout{s}")
                     for s in range(NSPL)]

            # ---------------- phase 1: h1_pre partial + AllReduce ----------
            with tc.tile_pool(name="p1nb", bufs=3) as p1nb, \
                 tc.tile_pool(name="p1w", bufs=3) as p1w:
                psA = pmps.tile([128, 1024], F32, name="mps", tag="mps")
                for c in range(30):
                    r0 = c * 125
                    nb_t = p1nb.tile([125, B], BF16, name="nb_t")
                    nc.sync.dma_start(nb_t[:], nbT[r0:r0 + 125, :])
                    w1_t = p1w.tile([125, TH], BF16, name="w1_t")
                    nc.sync.dma_start(w1_t[:], w1vT[r0:r0 + 125, :])
                    nc.tensor.matmul(psA[:, :400], nb_t[:], w1_t[:, :400],
                                     start=(c == 0), stop=(c == 29))
                    nc.tensor.matmul(psA[:, 400:TH], nb_t[:], w1_t[:, 400:],
                                     start=(c == 0), stop=(c == 29))
                h1preS = outer.tile([B, TH], F32)
                nc.vector.tensor_copy(h1preS[:], psA[:, :TH])
                nc.sync.dma_start(ar1_in[:], h1preS[:])
            nc.gpsimd.collective_compute(
                "AllReduce", OP.add,
                replica_groups=[list(range(NCORES))],
                ins=[ar1_in[:].opt()], outs=[ar1_out[:].opt()])

            # more persistent DMAs (needed a bit later)
            etaC_sb = outer.tile([B, TH], F32)
            nc.sync.dma_start(etaC_sb[:], etaC[:])
            w2T_sb = outer.tile([128, 7, TH], BF16)
            for j in range(7):
                nc.sync.dma_start(w2T_sb[:KJ[j], j, :],
                                  w2T[j * 128:j * 128 + KJ[j], :])
            wmuls_sb = outer.tile([128, 7, 2 * K], BF16)
            for j in range(7):
                nc.sync.dma_start(wmuls_sb[:KJ[j], j, :],
                                  wmulsT[j * 128:j * 128 + KJ[j], :])
            b2R_sb = outer.tile([128, 7], F32)
            nc.sync.dma_start(b2R_sb[:], b2R[:])
            bmuls_sb = outer.tile([B, 2 * K], F32)
            nc.sync.dma_start(bmuls_sb[:], bmulsB[:])
            etaTD_sb = outer.tile([B, K], F32)
            nc.sync.dma_start(etaTD_sb[:], etaTD[:])
            epsTH_sb = outer.tile([B, K], F32)
            nc.sync.dma_start(epsTH_sb[:], epsTH[:])

            # canary: rho as seen on device (bf16 DMA integrity check)
            can = outer.tile([128, 64], F32)
            nc.vector.tensor_copy(can[:], rho_sb[:, 0, 0:64])
            nc.sync.dma_start(canOut[:], can[:])

            exl = [None] * NSPL
            zr = [None] * NSPL

            def einsum_split(s):
                alph = sp.tile([128, 3, MS * 128], BF16, name="alph")
                c0 = s * MS * 128
                for kc in range(3):
                    nc.sync.dma_start(
                        alph[:EK[kc], kc, :],
                        alphasT[kc * 128:kc * 128 + EK[kc],
                                c0:c0 + MS * 128])
                ex = sp.tile([128, MS, VS], BF16, name="exl")
                exl[s] = ex
                zsp = sp.tile([128, MS, 4], F32, name="zsp")
                for ml in range(MS):
                    for n4 in range(4):
                        w = NW4[n4]
                        n0 = n4 * 1024
                        ps = peps.tile([128, 1024], F32, name="eps")
                        for h in range(2):
                            hw = min(512, w - h * 512)
                            if hw <= 0:
                                continue
                            for kc in range(3):
                                nc.tensor.matmul(
                                    ps[:, h * 512:h * 512 + hw],
                                    alph[:EK[kc], kc, ml * 128:(ml + 1) * 128],
                                    rho_sb[:EK[kc], kc,
                                           n0 + h * 512:n0 + h * 512 + hw],
                                    start=(kc == 0), stop=(kc == 2))
                        nc.scalar.activation(
                            ex[:, ml, n0:n0 + w], ps[:, :w], AF.Exp,
                            accum_out=zsp[:, ml, n4:n4 + 1])
                zred = sp.tile([128, MS], F32, name="zred")
                nc.vector.reduce_sum(zred[:], zsp[:],
                                     axis=mybir.AxisListType.X)
                nc.sync.dma_start(
                    z_in[s][:].rearrange("(a b) -> b a", b=128), zred[:])
                nc.gpsimd.collective_compute(
                    "AllReduce", OP.add,
                    replica_groups=[list(range(NCORES))],
                    ins=[z_in[s][:].opt()], outs=[z_out[s][:].opt()])
                zrs = sp.tile([128, MS], F32, name="zrs")
                nc.sync.dma_start(
                    zrs[:], z_out[s][:].rearrange("(a b) -> b a", b=128))
                zrec = sp.tile([128, MS], F32, name="zrec")
                nc.vector.reciprocal(zrec[:], zrs[:])
                zr[s] = zrec

            def mix_split(s, thetaT_bf):
                g = sp.tile([128, MS, B], BF16, name="gsp")
                for ml in range(MS):
                    m = s * MS + ml
                    rep = pmps.tile([128, 1024], F32, name="mps", tag="mps")
                    nc.tensor.matmul(rep[:, :B], psel_sb[:, m, :],
                                     thetaT_bf[:], start=True, stop=True)
                    gtmp = wrk.tile([128, B], F32, name="gtmp")
                    nc.vector.tensor_scalar_mul(gtmp[:], rep[:, :B],
                                                zr[s][:, ml:ml + 1])
                    nc.vector.tensor_mul(g[:, ml, :], gtmp[:],
                                         masks_sb[:, m, :])
                for n4 in range(4):
                    w = NW4[n4]
                    n0 = n4 * 1024
                    mps = pmps.tile([128, 1024], F32, name="mps", tag="mps")
                    for h in range(2):
                        hw = min(512, w - h * 512)
                        if hw <= 0:
                            continue
                        for ml in range(MS):
                            nc.tensor.matmul(
                                mps[:, h * 512:h * 512 + hw], g[:, ml, :],
                                exl[s][:, ml, n0 + h * 512:n0 + h * 512 + hw],
                                start=(ml == 0), stop=(ml == MS - 1))
                    nc.vector.tensor_add(mixacc[:, n0:n0 + w],
                                         mixacc[:, n0:n0 + w], mps[:, :w])

            # splits 0 and 1 einsum first (AR overlap), then MLP, then
            # software-pipelined G/mix against later splits.
            einsum_split(0)
            einsum_split(1)

            # ---------------- theta MLP ----------------
            h1pre_sb = outer.tile([B, TH], F32)
            nc.sync.dma_start(h1pre_sb[:], ar1_out[:])
            hsum = outer.tile([B, TH], F32)
            nc.vector.tensor_add(hsum[:], h1pre_sb[:], etaC_sb[:])
            h1 = outer.tile([B, TH], F32)
            nc.scalar.activation(h1[:], hsum[:], AF.Relu)
            h1T = outer.tile([128, 7, B], BF16)
            for j in range(7):
                wj = KJ[j]
                pt = pmps.tile([128, 1024], F32, name="mps", tag="mps")
                nc.tensor.transpose(pt[:wj, :128],
                                    h1[:, j * 128:j * 128 + wj], ident[:])
                nc.vector.tensor_copy(h1T[:wj, j, :], pt[:wj, :128])
            h2T = outer.tile([128, 7, B], BF16)
            for jo in range(7):
                wjo = KJ[jo]
                psH = pmps.tile([128, 1024], F32, name="mps", tag="mps")
                for ji in range(7):
                    nc.tensor.matmul(
                        psH[:wjo, :128],
                        w2T_sb[:KJ[ji], ji, jo * 128:jo * 128 + wjo],
                        h1T[:KJ[ji], ji, :],
                        start=(ji == 0), stop=(ji == 6))
                nc.scalar.activation(h2T[:wjo, jo, :], psH[:wjo, :128],
                                     AF.Relu, bias=b2R_sb[:wjo, jo:jo + 1])
            psM = pmps.tile([128, 1024], F32, name="mps", tag="mps")
            for ji in range(7):
                nc.tensor.matmul(psM[:, :2 * K], h2T[:KJ[ji], ji, :],
                                 wmuls_sb[:KJ[ji], ji, :],
                                 start=(ji == 0), stop=(ji == 6))
            muls = outer.tile([B, 2 * K], F32)
            nc.vector.tensor_add(muls[:], psM[:, :2 * K], bmuls_sb[:])
            mu = muls[:, :K]
            ls = muls[:, K:]
            sd = outer.tile([B, K], F32)
            nc.scalar.activation(sd[:], ls, AF.Exp, scale=0.5)
            ez0 = outer.tile([B, K], F32)
            nc.vector.tensor_mul(ez0[:], epsTH_sb[:], sd[:])
            zt = outer.tile([B, K], F32)
            nc.vector.tensor_add(zt[:], mu, ez0[:])
            zm = outer.tile([B, 1], F32)
            nc.vector.reduce_max(zm[:], zt[:], axis=mybir.AxisListType.X,
                                 negate=True)
            et = outer.tile([B, K], F32)
            se = outer.tile([B, 1], F32)
            nc.scalar.activation(et[:], zt[:], AF.Exp, bias=zm[:],
                                 accum_out=se[:])
            rse = outer.tile([B, 1], F32)
            nc.vector.reciprocal(rse[:], se[:])
            theta = outer.tile([B, K], F32)
            nc.vector.tensor_scalar_mul(theta[:], et[:], rse[:])

            # kl_theta (replicated on every core; host uses core 0)
            sd2 = outer.tile([B, K], F32)
            nc.vector.tensor_mul(sd2[:], sd[:], sd[:])
            dd = outer.tile([B, K], F32)
            nc.vector.tensor_sub(dd[:], mu, etaTD_sb[:])
            dd2 = outer.tile([B, K], F32)
            nc.vector.tensor_mul(dd2[:], dd[:], dd[:])
            uu = outer.tile([B, K], F32)
            sA = outer.tile([B, 1], F32)
            nc.vector.scalar_tensor_tensor(uu[:], dd2[:], 1.0, sd2[:],
                                           op0=OP.bypass, op1=OP.add,
                                           accum_out=sA[:])
            sB_ = outer.tile([B, 1], F32)
            nc.vector.reduce_sum(sB_[:], ls, axis=mybir.AxisListType.X)
            q1 = outer.tile([B, 1], F32)
            nc.vector.tensor_scalar(q1[:], sA[:], float(RKL), -float(K) * 0.5,
                                    op0=OP.mult, op1=OP.add)
            q2 = outer.tile([B, 1], F32)
            nc.vector.tensor_scalar_mul(q2[:], sB_[:], 0.5)
            klth = outer.tile([B, 1], F32)
            nc.vector.tensor_sub(klth[:], q1[:], q2[:])
            nc.sync.dma_start(klthOut[:], klth[:])

            # thetaT (bf16) for the per-split Psel matmuls
            thzp = outer.tile([128, 128], F32)
            nc.vector.memset(thzp[:], 0.0)
            nc.vector.tensor_copy(thzp[:, :K], theta[:])
            ptT = pmps.tile([128, 1024], F32, name="mps", tag="mps")
            nc.tensor.transpose(ptT[:, :128], thzp[:], ident[:])
            thetaT_bf = outer.tile([K, B], BF16)
            nc.vector.tensor_copy(thetaT_bf[:], ptT[:K, :128])

            # ---------------- pipelined einsum / mix ----------------
            mix_split(0, thetaT_bf)
            einsum_split(2)
            mix_split(1, thetaT_bf)
            einsum_split(3)
            mix_split(2, thetaT_bf)
            mix_split(3, thetaT_bf)

            # ---------------- nll ----------------
            nllp = outer.tile([B, 8], F32)
            NW8 = [512] * 7 + [166]
            for n8 in range(8):
                w = NW8[n8]
                n0 = n8 * 512
                bws = wrk.tile([128, 512], F32, name="bws")
                nc.sync.dma_start(bws[:, :w], bowsS[:, n0:n0 + w])
                lnm = wrk.tile([128, 512], F32, name="lnm")
                nc.scalar.activation(lnm[:, :w], mixacc[:, n0:n0 + w],
                                     AF.Ln, bias=eps6[:])
                junk = wrk.tile([128, 512], F32, name="junk")
                nc.vector.scalar_tensor_tensor(
                    junk[:, :w], lnm[:, :w], 1.0,
                    bws[:, :w],
                    op0=OP.bypass, op1=OP.mult,
                    accum_out=nllp[:, n8:n8 + 1])
            nsum = outer.tile([B, 1], F32)
            nc.vector.reduce_sum(nsum[:], nllp[:], axis=mybir.AxisListType.X,
                                 negate=True)
            nc.sync.dma_start(nllOut[:], nsum[:])

    nc.compile()
    return nc


# ---------------------------------------------------------------------------
# host-side small sequential chains (fp32 numpy)
# ---------------------------------------------------------------------------

def _sigmoid(x):
    with np.errstate(over="ignore"):
        return (1.0 / (1.0 + np.exp(-x))).astype(np.float32)


def _kl_np(qm, qls, pm, pls):
    return 0.5 * np.sum(
        (np.exp(qls) + (qm - pm) ** 2) / (np.exp(pls) + 1e-6)
        - 1.0 + pls - qls, axis=-1, dtype=np.float32)


def _host_chains(inp):
    f = np.float32
    mu_a = np.asarray(inp["mu_q_alpha"], f).transpose(1, 0, 2)
    ls_a = np.asarray(inp["logsigma_q_alpha"], f).transpose(1, 0, 2)
    eps_a = np.asarray(inp["eps_alpha"], f)
    logdelta = f(np.log(f(DELTA)))
    alphas = (mu_a + eps_a * np.exp(0.5 * ls_a)).astype(f)
    kl_alpha = f(_kl_np(mu_a[0], ls_a[0], f(0.0), f(0.0)).sum()
                 + _kl_np(mu_a[1:], ls_a[1:], alphas[:-1], logdelta).sum())

    rnn_inp = np.asarray(inp["rnn_inp"], f)
    Wmap = np.asarray(inp["Wmap"], f)
    bmap = np.asarray(inp["bmap"], f)
    out = (rnn_inp @ Wmap.T + bmap).astype(f)
    Wih = np.asarray(inp["lstm_Wih"], f)
    Whh = np.asarray(inp["lstm_Whh"], f)
    bih = np.asarray(inp["lstm_bih"], f)
    bhh = np.asarray(inp["lstm_bhh"], f)
    for l in range(L):
        h = np.zeros(H, f)
        c = np.zeros(H, f)
        pre = (out @ Wih[l].T + (bih[l] + bhh[l])).astype(f)
        ys = np.empty((T, H), f)
        for t in range(T):
            g = pre[t] + Whh[l] @ h
            i_, f_, g_, o_ = np.split(g, 4)
            c = _sigmoid(f_) * c + _sigmoid(i_) * np.tanh(g_)
            h = (_sigmoid(o_) * np.tanh(c)).astype(f)
            ys[t] = h
        out = ys
    Wmu_e = np.asarray(inp["Wmu_e"], f)
    bmu_e = np.asarray(inp["bmu_e"], f)
    Wls_e = np.asarray(inp["Wls_e"], f)
    bls_e = np.asarray(inp["bls_e"], f)
    eps_eta = np.asarray(inp["eps_eta"], f)
    inp0 = np.concatenate([out[0], np.zeros(K, f)])
    mu0 = Wmu_e @ inp0 + bmu_e
    ls0 = Wls_e @ inp0 + bls_e
    eta = mu0 + eps_eta[0] * np.exp(0.5 * ls0)
    kl_eta = _kl_np(mu0, ls0, f(0.0), f(0.0))
    etas = np.empty((T, K), f)
    etas[0] = eta
    for t in range(1, T):
        it = np.concatenate([out[t], eta])
        mu_t = Wmu_e @ it + bmu_e
        ls_t = Wls_e @ it + bls_e
        eta = (mu_t + eps_eta[t] * np.exp(0.5 * ls_t)).astype(f)
        kl_eta = kl_eta + _kl_np(mu_t, ls_t, etas[t - 1], logdelta)
        etas[t] = eta
    return alphas, f(kl_alpha), etas, f(kl_eta)


def kernel(**inputs):
    f = np.float32
    if "nc" not in _CACHE:
        _CACHE["nc"] = _build_program()
    nc = _CACHE["nc"]

    bows = np.asarray(inputs["bows"], f)
    nb = np.asarray(inputs["normalized_bows"], f)
    times = np.asarray(inputs["times"]).astype(np.int64)
    num_docs = float(np.asarray(inputs["num_docs"]))
    W1 = np.asarray(inputs["W1"], f)
    b1 = np.asarray(inputs["b1"], f)
    W2 = np.asarray(inputs["W2"], f)
    b2 = np.asarray(inputs["b2"], f)
    Wmu_t = np.asarray(inputs["Wmu_t"], f)
    bmu_t = np.asarray(inputs["bmu_t"], f)
    Wls_t = np.asarray(inputs["Wls_t"], f)
    bls_t = np.asarray(inputs["bls_t"], f)
    rho = np.asarray(inputs["rho"], f)
    eps_theta = np.asarray(inputs["eps_theta"], f)

    alphas, kl_alpha, etas, kl_eta = _host_chains(inputs)
    eta_td = etas[times]                                   # [B, K]
    etaC = (eta_td @ W1[:, V:].T + b1).astype(f)           # [B, TH]

    # padded [E, TKP] alphas
    ap = np.zeros((TKP, E), f)
    ap[:TK] = alphas.reshape(TK, E)
    alphasT = np.ascontiguousarray(ap.T).astype(BFNP)

    # mask/psel over padded rows
    r_pm = np.arange(128)[:, None] + 128 * np.arange(MCH)[None, :]  # [128,24]
    t_r = r_pm // K
    k_r = r_pm % K
    valid = r_pm < TK
    maskP = ((times[None, None, :] == t_r[:, :, None])
             & valid[:, :, None]).astype(BFNP)             # [128,24,B]
    pselP = ((np.arange(K)[:, None, None] == k_r.T[None, :, :])
             & valid.T[None, :, :]).astype(BFNP)           # [K,24,128]

    w2T = np.ascontiguousarray(W2.T).astype(BFNP)
    wmulsT = np.ascontiguousarray(
        np.concatenate([Wmu_t, Wls_t], axis=0).T).astype(BFNP)
    pad = np.zeros(896, f)
    pad[:TH] = b2
    b2R = np.ascontiguousarray(pad.reshape(7, 128).T)
    bmulsB = np.ascontiguousarray(
        np.broadcast_to(np.concatenate([bmu_t, bls_t]).astype(f), (B, 2 * K)))

    in_maps = []
    for c in range(NCORES):
        sl = slice(c * VS, (c + 1) * VS)
        in_maps.append({
            "nbT": np.ascontiguousarray(nb[:, sl].T).astype(BFNP),
            "w1vT": np.ascontiguousarray(W1[:, sl].T).astype(BFNP),
            "rhoT": np.ascontiguousarray(rho[sl, :].T).astype(BFNP),
            "alphasT": alphasT,
            "bowsS": np.ascontiguousarray(bows[:, sl]),
            "maskB": np.ascontiguousarray(maskP.reshape(128, MCH * B)),
            "pselB": np.ascontiguousarray(pselP.reshape(K, MCH * 128)),
            "etaC": etaC,
            "etaTD": np.ascontiguousarray(eta_td.astype(f)),
            "epsTH": eps_theta,
            "w2T": w2T,
            "wmulsT": wmulsT,
            "b2R": b2R,
            "bmulsB": bmulsB,
        })

    global _LAST_IN_MAPS
    _LAST_IN_MAPS = in_maps
    res = bass_utils.run_bass_kernel_spmd(nc, in_maps,
                                          core_ids=list(range(NCORES)))
    _CACHE["res"] = res
    coeff = f(num_docs / B)
    nll_tot = f(sum(r["nllOut"].sum(dtype=np.float64) for r in res.results))
    nll_tot = f(nll_tot * coeff)
    klth_tot = f(res.results[0]["klthOut"].sum(dtype=np.float64) * coeff)
    nelbo = f(nll_tot + kl_alpha + kl_eta + klth_tot)
    return np.array([nelbo, nll_tot, kl_alpha, kl_eta, klth_tot], dtype=f)


# revision 11
# speedup vs baseline: 1.7183x; 1.7183x over previous
"""DETM nelbo kernel for 8 Trainium2 NeuronCores (v3).

Sharding: vocabulary V=30000 split 8 ways (3750/core). The device computes
the dominant beta/nll path (~97% of FLOPs):
 - logit[t,k,v] = alphas.rho per V-slice (bf16 matmuls, fp32 PSUM accum);
   exp(logit) kept RESIDENT in SBUF as bf16 (no DRAM round trip), processed
   in 4 splits of 6x128 TK rows with ping-pong buffers.
 - per-split Z partials AllReduced (3KB each), overlapped with the next
   split's einsum.
 - G[r, b] = theta[b, k(r)] * (times[b]==t(r)) / Z[r] built on device per
   split (Psel matmul + two DVE muls); mix[b,v] accumulated via PE matmuls
   into an SBUF f32 accumulator; nll = -sum bows*ln(mix+1e-6) per V-slice.
Replicated on the host in fp32 numpy: the small sequential chains (alpha
reparam + KLs, LSTM, eta chain) and the theta MLP + kl_theta (~6% of FLOPs,
but it would need a 400KB h1 AllReduce on device).
"""
import sys

if "/opt/trn_rl_repo" not in sys.path:
    sys.path.insert(0, "/opt/trn_rl_repo")

import numpy as np
import ml_dtypes

import concourse.bass as bass
import concourse.mybir as mybir
import concourse.tile as tile
from concourse import bacc, bass_utils

F32 = mybir.dt.float32
BF16 = mybir.dt.bfloat16
AF = mybir.ActivationFunctionType
OP = mybir.AluOpType
BFNP = ml_dtypes.bfloat16

V, K, E, T, B = 30000, 50, 300, 60, 128
TH, H, L = 800, 200, 3
NCORES = 8
VS = V // NCORES          # 3750
TK = T * K                # 3000
TKP = 3072                # padded to 24 chunks of 128
MCH = 24                  # TK chunks of 128 rows
NSPL = 4                  # splits
MS = MCH // NSPL          # 6 chunks per split
NW4 = [1024, 1024, 1024, 678]   # V-chunking of VS=3750
DELTA = 0.005

EK = [128, 128, 44]       # E=300 partition split

_CACHE = {}


def _build_program():
    nc = bacc.Bacc("TRN2", target_bir_lowering=False, debug=False,
                   num_devices=NCORES)

    def din(name, shape, dt=F32):
        return nc.dram_tensor(name, shape, dt, kind="ExternalInput").ap()

    rhoT = din("rhoT", [E, VS], BF16)
    alphasT = din("alphasT", [E, TKP], BF16)
    bowsS = din("bowsS", [B, VS])
    maskB = din("maskB", [128, MCH * B], BF16)
    pselB = din("pselB", [K, MCH * 128], BF16)
    thetaTB = din("thetaTB", [K, B], BF16)

    nllOut = nc.dram_tensor("nllOut", [B, 1], F32, kind="ExternalOutput").ap()

    with tile.TileContext(nc) as tc:
        with tc.tile_pool(name="outer", bufs=1) as outer, \
             tc.tile_pool(name="sp", bufs=2) as sp, \
             tc.tile_pool(name="wrk", bufs=2) as wrk, \
             tc.tile_pool(name="dramp", bufs=1, space="DRAM") as dram, \
             tc.tile_pool(name="peps", bufs=2, space="PSUM") as peps, \
             tc.tile_pool(name="pmps", bufs=2, space="PSUM") as pmps:

            mixacc = outer.tile([128, VS], F32)
            nc.vector.memset(mixacc[:], 0.0)
            eps6 = outer.tile([B, 1], F32)
            nc.vector.memset(eps6[:], 1e-6)

            rho_sb = outer.tile([128, 3, VS], BF16)
            for kc in range(3):
                nc.sync.dma_start(rho_sb[:EK[kc], kc, :],
                                  rhoT[kc * 128:kc * 128 + EK[kc], :])
            masks_sb = outer.tile([128, MCH, B], BF16)
            nc.sync.dma_start(masks_sb[:],
                              maskB[:].rearrange("p (m b) -> p m b", b=B))
            psel_sb = outer.tile([K, MCH, 128], BF16)
            nc.sync.dma_start(psel_sb[:],
                              pselB[:].rearrange("p (m b) -> p m b", b=128))
            thetaT_bf = outer.tile([K, B], BF16)
            nc.sync.dma_start(thetaT_bf[:], thetaTB[:])

            z_in = [dram.tile([MS * 128], F32, name=f"zin{s}")
                    for s in range(NSPL)]
            z_out = [dram.tile([MS * 128], F32, addr_space="Shared",
                               name=f"zout{s}")
                     for s in range(NSPL)]

            exl = [None] * NSPL
            zr = [None] * NSPL

            def einsum_split(s):
                alph = sp.tile([128, 3, MS * 128], BF16, name="alph")
                c0 = s * MS * 128
                for kc in range(3):
                    nc.sync.dma_start(
                        alph[:EK[kc], kc, :],
                        alphasT[kc * 128:kc * 128 + EK[kc],
                                c0:c0 + MS * 128])
                ex = sp.tile([128, MS, VS], BF16, name="exl")
                exl[s] = ex
                zsp = sp.tile([128, MS, 4], F32, name="zsp")
                for ml in range(MS):
                    for n4 in range(4):
                        w = NW4[n4]
                        n0 = n4 * 1024
                        ps = peps.tile([128, 1024], F32, name="eps")
                        for h in range(2):
                            hw = min(512, w - h * 512)
                            if hw <= 0:
                                continue
                            for kc in range(3):
                                nc.tensor.matmul(
                                    ps[:, h * 512:h * 512 + hw],
                                    alph[:EK[kc], kc, ml * 128:(ml + 1) * 128],
                                    rho_sb[:EK[kc], kc,
                                           n0 + h * 512:n0 + h * 512 + hw],
                                    start=(kc == 0), stop=(kc == 2))
                        nc.scalar.activation(
                            ex[:, ml, n0:n0 + w], ps[:, :w], AF.Exp,
                            accum_out=zsp[:, ml, n4:n4 + 1])
                zred = sp.tile([128, MS], F32, name="zred")
                nc.vector.reduce_sum(zred[:], zsp[:],
                                     axis=mybir.AxisListType.X)
                nc.sync.dma_start(
                    z_in[s][:].rearrange("(a b) -> b a", b=128), zred[:])
                nc.gpsimd.collective_compute(
                    "AllReduce", OP.add,
                    replica_groups=[list(range(NCORES))],
                    ins=[z_in[s][:].opt()], outs=[z_out[s][:].opt()])
                zrs = sp.tile([128, MS], F32, name="zrs")
                nc.sync.dma_start(
                    zrs[:], z_out[s][:].rearrange("(a b) -> b a", b=128))
                zrec = sp.tile([128, MS], F32, name="zrec")
                nc.vector.reciprocal(zrec[:], zrs[:])
                zr[s] = zrec

            def mix_split(s):
                g = sp.tile([128, MS, B], BF16, name="gsp")
                for ml in range(MS):
                    m = s * MS + ml
                    rep = pmps.tile([128, 1024], F32, name="mps", tag="mps")
                    nc.tensor.matmul(rep[:, :B], psel_sb[:, m, :],
                                     thetaT_bf[:], start=True, stop=True)
                    gtmp = wrk.tile([128, B], F32, name="gtmp")
                    nc.vector.tensor_scalar_mul(gtmp[:], rep[:, :B],
                                                zr[s][:, ml:ml + 1])
                    nc.vector.tensor_mul(g[:, ml, :], gtmp[:],
                                         masks_sb[:, m, :])
                for n4 in range(4):
                    w = NW4[n4]
                    n0 = n4 * 1024
                    mps = pmps.tile([128, 1024], F32, name="mps", tag="mps")
                    for h in range(2):
                        hw = min(512, w - h * 512)
                        if hw <= 0:
                            continue
                        for ml in range(MS):
                            nc.tensor.matmul(
                                mps[:, h * 512:h * 512 + hw], g[:, ml, :],
                                exl[s][:, ml, n0 + h * 512:n0 + h * 512 + hw],
                                start=(ml == 0), stop=(ml == MS - 1))
                    nc.vector.tensor_add(mixacc[:, n0:n0 + w],
                                         mixacc[:, n0:n0 + w], mps[:, :w])

            einsum_split(0)
            einsum_split(1)
            mix_split(0)
            einsum_split(2)
            mix_split(1)
            einsum_split(3)
            mix_split(2)
            mix_split(3)

            # ---------------- nll ----------------
            nllp = outer.tile([B, 8], F32)
            NW8 = [512] * 7 + [166]
            for n8 in range(8):
                w = NW8[n8]
                n0 = n8 * 512
                bws = wrk.tile([128, 512], F32, name="bws")
                nc.sync.dma_start(bws[:, :w], bowsS[:, n0:n0 + w])
                lnm = wrk.tile([128, 512], F32, name="lnm")
                nc.scalar.activation(lnm[:, :w], mixacc[:, n0:n0 + w],
                                     AF.Ln, bias=eps6[:])
                junk = wrk.tile([128, 512], F32, name="junk")
                nc.vector.scalar_tensor_tensor(
                    junk[:, :w], lnm[:, :w], 1.0,
                    bws[:, :w],
                    op0=OP.bypass, op1=OP.mult,
                    accum_out=nllp[:, n8:n8 + 1])
            nsum = outer.tile([B, 1], F32)
            nc.vector.reduce_sum(nsum[:], nllp[:], axis=mybir.AxisListType.X,
                                 negate=True)
            nc.sync.dma_start(nllOut[:], nsum[:])

    nc.compile()
    return nc


# ---------------------------------------------------------------------------
# host-side small sequential chains + theta MLP (fp32 numpy)
# ---------------------------------------------------------------------------

def _sigmoid(x):
    with np.errstate(over="ignore"):
        return (1.0 / (1.0 + np.exp(-x))).astype(np.float32)


def _kl_np(qm, qls, pm, pls):
    return 0.5 * np.sum(
        (np.exp(qls) + (qm - pm) ** 2) / (np.exp(pls) + 1e-6)
        - 1.0 + pls - qls, axis=-1, dtype=np.float32)


def _host_chains(inp):
    f = np.float32
    mu_a = np.asarray(inp["mu_q_alpha"], f).transpose(1, 0, 2)
    ls_a = np.asarray(inp["logsigma_q_alpha"], f).transpose(1, 0, 2)
    eps_a = np.asarray(inp["eps_alpha"], f)
    logdelta = f(np.log(f(DELTA)))
    alphas = (mu_a + eps_a * np.exp(0.5 * ls_a)).astype(f)
    kl_alpha = f(_kl_np(mu_a[0], ls_a[0], f(0.0), f(0.0)).sum()
                 + _kl_np(mu_a[1:], ls_a[1:], alphas[:-1], logdelta).sum())

    rnn_inp = np.asarray(inp["rnn_inp"], f)
    Wmap = np.asarray(inp["Wmap"], f)
    bmap = np.asarray(inp["bmap"], f)
    out = (rnn_inp @ Wmap.T + bmap).astype(f)
    Wih = np.asarray(inp["lstm_Wih"], f)
    Whh = np.asarray(inp["lstm_Whh"], f)
    bih = np.asarray(inp["lstm_bih"], f)
    bhh = np.asarray(inp["lstm_bhh"], f)
    for l in range(L):
        h = np.zeros(H, f)
        c = np.zeros(H, f)
        pre = (out @ Wih[l].T + (bih[l] + bhh[l])).astype(f)
        ys = np.empty((T, H), f)
        for t in range(T):
            g = pre[t] + Whh[l] @ h
            i_, f_, g_, o_ = np.split(g, 4)
            c = _sigmoid(f_) * c + _sigmoid(i_) * np.tanh(g_)
            h = (_sigmoid(o_) * np.tanh(c)).astype(f)
            ys[t] = h
        out = ys
    Wmu_e = np.asarray(inp["Wmu_e"], f)
    bmu_e = np.asarray(inp["bmu_e"], f)
    Wls_e = np.asarray(inp["Wls_e"], f)
    bls_e = np.asarray(inp["bls_e"], f)
    eps_eta = np.asarray(inp["eps_eta"], f)
    inp0 = np.concatenate([out[0], np.zeros(K, f)])
    mu0 = Wmu_e @ inp0 + bmu_e
    ls0 = Wls_e @ inp0 + bls_e
    eta = mu0 + eps_eta[0] * np.exp(0.5 * ls0)
    kl_eta = _kl_np(mu0, ls0, f(0.0), f(0.0))
    etas = np.empty((T, K), f)
    etas[0] = eta
    for t in range(1, T):
        it = np.concatenate([out[t], eta])
        mu_t = Wmu_e @ it + bmu_e
        ls_t = Wls_e @ it + bls_e
        eta = (mu_t + eps_eta[t] * np.exp(0.5 * ls_t)).astype(f)
        kl_eta = kl_eta + _kl_np(mu_t, ls_t, etas[t - 1], logdelta)
        etas[t] = eta
    return alphas, f(kl_alpha), etas, f(kl_eta)


def kernel(**inputs):
    f = np.float32
    if "nc" not in _CACHE:
        _CACHE["nc"] = _build_program()
    nc = _CACHE["nc"]

    bows = np.asarray(inputs["bows"], f)
    nb = np.asarray(inputs["normalized_bows"], f)
    times = np.asarray(inputs["times"]).astype(np.int64)
    num_docs = float(np.asarray(inputs["num_docs"]))
    W1 = np.asarray(inputs["W1"], f)
    b1 = np.asarray(inputs["b1"], f)
    W2 = np.asarray(inputs["W2"], f)
    b2 = np.asarray(inputs["b2"], f)
    Wmu_t = np.asarray(inputs["Wmu_t"], f)
    bmu_t = np.asarray(inputs["bmu_t"], f)
    Wls_t = np.asarray(inputs["Wls_t"], f)
    bls_t = np.asarray(inputs["bls_t"], f)
    rho = np.asarray(inputs["rho"], f)
    eps_theta = np.asarray(inputs["eps_theta"], f)

    alphas, kl_alpha, etas, kl_eta = _host_chains(inputs)
    eta_td = etas[times]                                   # [B, K]

    # theta MLP + kl_theta (host, fp32 — replicates reference exactly)
    h1 = np.maximum(nb @ W1[:, :V].T + eta_td @ W1[:, V:].T + b1, 0).astype(f)
    h2 = np.maximum(h1 @ W2.T + b2, 0).astype(f)
    mu_th = (h2 @ Wmu_t.T + bmu_t).astype(f)
    ls_th = (h2 @ Wls_t.T + bls_t).astype(f)
    zth = mu_th + eps_theta * np.exp(0.5 * ls_th).astype(f)
    ezt = np.exp(zth - zth.max(1, keepdims=True)).astype(f)
    theta = (ezt / ezt.sum(1, keepdims=True)).astype(f)
    klth = _kl_np(mu_th, ls_th, eta_td, f(0.0))

    # padded [E, TKP] alphas
    ap = np.zeros((TKP, E), f)
    ap[:TK] = alphas.reshape(TK, E)
    alphasT = np.ascontiguousarray(ap.T).astype(BFNP)

    # mask/psel over padded rows
    r_pm = np.arange(128)[:, None] + 128 * np.arange(MCH)[None, :]  # [128,24]
    t_r = r_pm // K
    k_r = r_pm % K
    valid = r_pm < TK
    maskP = ((times[None, None, :] == t_r[:, :, None])
             & valid[:, :, None]).astype(BFNP)             # [128,24,B]
    pselP = ((np.arange(K)[:, None, None] == k_r.T[None, :, :])
             & valid.T[None, :, :]).astype(BFNP)           # [K,24,128]
    thetaTB = np.ascontiguousarray(theta.T).astype(BFNP)   # [K,B]

    in_maps = []
    for c in range(NCORES):
        sl = slice(c * VS, (c + 1) * VS)
        in_maps.append({
            "rhoT": np.ascontiguousarray(rho[sl, :].T).astype(BFNP),
            "alphasT": alphasT,
            "bowsS": np.ascontiguousarray(bows[:, sl]),
            "maskB": np.ascontiguousarray(maskP.reshape(128, MCH * B)),
            "pselB": np.ascontiguousarray(pselP.reshape(K, MCH * 128)),
            "thetaTB": thetaTB,
        })

    global _LAST_IN_MAPS
    _LAST_IN_MAPS = in_maps
    res = bass_utils.run_bass_kernel_spmd(nc, in_maps,
                                          core_ids=list(range(NCORES)))
    _CACHE["res"] = res
    coeff = f(num_docs / B)
    nll_tot = f(sum(r["nllOut"].sum(dtype=np.float64) for r in res.results))
    nll_tot = f(nll_tot * coeff)
    klth_tot = f(klth.sum(dtype=np.float64) * coeff)
    nelbo = f(nll_tot + kl_alpha + kl_eta + klth_tot)
    return np.array([nelbo, nll_tot, kl_alpha, kl_eta, klth_tot], dtype=f)
